# revision 34
# baseline (speedup 1.0000x reference)
"""Trainium2 Bass kernel for DifferentiableLogicLayer (scatter_memory).

Math (see reference):
    gates    = sigmoid(gate_logits)                       [512]
    pen_1d   = scatter_add over E=1M pairs: 0.5*gates[rule] at token  [V]
    modified = logits - pen_1d[None, :]
    coverage = -(gates * firing).sum() / max(n_firing, 1)

Device strategy: the scatter-add is re-expressed as a dense matvec.
The (rule, token) pairs are binned on the host (pure integer index
preprocessing, per the vocab-shard sharding hint) into a count matrix
C[rule, v]; counts are tiny integers, exact in fp8.  Each of the 8
cores owns a 16000-wide vocab shard and computes

    pen[v] = sum_r C[r, v] * (0.5 * sigmoid(gate_logits[r]))

on the PE as 8 accumulating matmuls per 500-wide vocab tile (4 K-blocks
of 128 rules x {hi, lo} fp16 split of the gates for fp32-level
accuracy), with the 64 identical lhsT columns providing the batch
broadcast for free.  DVE then forms modified = logits - pen and the
penalties copy; outputs stream back over the ACT HWDGE queue.
"""
import numpy as np
import ml_dtypes

import concourse.bacc as bacc
import concourse.tile as tile
import concourse.mybir as mybir
from concourse.bass_utils import run_bass_kernel_spmd

F32 = mybir.dt.float32
F16 = mybir.dt.float16
F8 = mybir.dt.float8e4

B = 64             # batch
V = 128000         # vocab
R = 512            # rules
NCORES = 8
VSHARD = V // NCORES   # 16000 vocab per core
import os
VT = 500           # vocab tile (psum free dim)
G = int(os.environ.get("KERNEL_G", "4"))   # vocab tiles per DMA group
NG = VSHARD // (VT * G)   # groups per core
# 1: add the fp16 "lo" residual matmul stream (error ~3e-7 rel); 0: fp16
# gates only (error ~3e-5 rel, half the PE work).
USE_LO = os.environ.get("KERNEL_LO", "1") == "1"
KB = R // 128      # 4 rule blocks of 128

PENALTY_LAMBDA = 0.5


def _build_program():
    nc = bacc.Bacc("TRN2", target_bir_lowering=False, debug=False,
                   num_devices=NCORES)

    gl_d = nc.dram_tensor("gl", [R], F32, kind="ExternalInput").ap()
    fs_d = nc.dram_tensor("fs", [R], F32, kind="ExternalInput").ap()
    # counts, host-laid-out as [group][partition][vt-in-group][kb][v]
    ct_d = nc.dram_tensor("ct", [NG, 128, G * KB * VT], F8,
                          kind="ExternalInput").ap()
    # logits/outputs in host-stacked layout [group][128][half-width]:
    # partition p = h*64+b holds batch b, cols of half h of the group.
    logits_d = nc.dram_tensor("logits", [NG, 128, G * VT // 2], F32,
                              kind="ExternalInput").ap()
    mod_d = nc.dram_tensor("mod", [NG, 128, G * VT // 2], F32,
                           kind="ExternalOutput").ap()
    pen_d = nc.dram_tensor("pen", [NG, 2, G * VT // 2], F32,
                           kind="ExternalOutput").ap()
    cov_d = nc.dram_tensor("cov", [1], F32, kind="ExternalOutput").ap()

    with tile.TileContext(nc) as tc:
        with (
            tc.tile_pool(name="const", bufs=1) as const_pool,
            tc.tile_pool(name="cts", bufs=6) as ct_pool,
            tc.tile_pool(name="lgt", bufs=6) as lgt_pool,
            tc.tile_pool(name="outs", bufs=3) as out_pool,
            tc.tile_pool(name="psum", bufs=8, space="PSUM") as psum_pool,
        ):
            # ---- tiny parameter loads first (they gate the gates-prep chain
            # that every matmul depends on), then the big streaming prefetch.
            gl_sb = const_pool.tile([128, KB], F32)
            nc.sync.dma_start(gl_sb[:], gl_d.rearrange("(p k) -> p k", p=128))
            glr_sb = const_pool.tile([1, R], F32)
            nc.sync.dma_start(glr_sb[:], gl_d.rearrange("(o r) -> o r", o=1))
            fsr_sb = const_pool.tile([1, R], F32)
            nc.sync.dma_start(fsr_sb[:], fs_d.rearrange("(o r) -> o r", o=1))

            # Stacked layout: a group covers G*VT vocab cols; cols are split
            # into halves X|Y. SBUF partitions 0-63 hold (batch, X-cols),
            # 64-127 hold (batch, Y-cols) so DMA/DVE run all 128 partitions.
            GW = G * VT       # group width in vocab cols
            HW_ = GW // 2     # half width

            ct_tiles = {}
            lg_tiles = {}

            def issue_loads(gi):
                ct_sb = ct_pool.tile([128, G * KB * VT], F8, tag="ct")
                qw = KB * VT
                for q in range(G):
                    nc.sync.dma_start(ct_sb[:, q * qw:(q + 1) * qw],
                                      ct_d[gi, :, q * qw:(q + 1) * qw])
                lg_sb = lgt_pool.tile([128, HW_], F32, tag="lg")
                nc.scalar.dma_start(lg_sb[:], logits_d[gi])
                ct_tiles[gi] = ct_sb
                lg_tiles[gi] = lg_sb

            for gi in range(min(4, NG)):
                issue_loads(gi)

            # ---- gates prep: rule r lives at (partition p, col k), r = p*KB+k
            g_sb = const_pool.tile([128, KB], F32)
            nc.scalar.activation(g_sb[:], gl_sb[:],
                                 mybir.ActivationFunctionType.Sigmoid)
            g2_sb = const_pool.tile([128, KB], F32)
            nc.vector.tensor_scalar_mul(g2_sb[:], g_sb[:], PENALTY_LAMBDA)
            hi16 = const_pool.tile([128, KB], F16)
            nc.vector.tensor_copy(hi16[:], g2_sb[:])
            hi32 = const_pool.tile([128, KB], F32)
            nc.vector.tensor_copy(hi32[:], hi16[:])
            lo32 = const_pool.tile([128, KB], F32)
            nc.vector.tensor_sub(lo32[:], g2_sb[:], hi32[:])

            ones = const_pool.tile([128, B], F32)
            nc.vector.memset(ones[:], 1.0)
            lhsT_hi = const_pool.tile([128, KB * B], F16)
            lhsT_lo = const_pool.tile([128, KB * B], F16)
            for kb in range(KB):
                nc.vector.tensor_scalar_mul(
                    lhsT_hi[:, kb * B:(kb + 1) * B], ones[:],
                    hi32[:, kb:kb + 1])
                nc.vector.tensor_scalar_mul(
                    lhsT_lo[:, kb * B:(kb + 1) * B], ones[:],
                    lo32[:, kb:kb + 1])

            # ---- coverage loss (tiny; every core computes it)
            gr_sb = const_pool.tile([1, R], F32)
            nc.scalar.activation(gr_sb[:], glr_sb[:],
                                 mybir.ActivationFunctionType.Sigmoid)
            prod_sb = const_pool.tile([1, R], F32)
            nc.vector.tensor_mul(prod_sb[:], gr_sb[:], fsr_sb[:])
            cov_sb = const_pool.tile([1, 1], F32)
            nc.vector.tensor_reduce(cov_sb[:], prod_sb[:],
                                    mybir.AxisListType.X, mybir.AluOpType.add)
            nc.scalar.dma_start(cov_d, cov_sb[0, :])

            # ---- main loop over vocab groups
            for gi in range(NG):
                if gi + 4 < NG:
                    issue_loads(gi + 4)
                ct_sb = ct_tiles.pop(gi)
                lg_sb = lg_tiles.pop(gi)

                mod_sb = out_pool.tile([128, HW_], F32, tag="mod")
                pen_sb = out_pool.tile([128, HW_], F32, tag="pen")
                nhalf = G // 2
                for s in range(nhalf):
                    ps = psum_pool.tile([128, VT], F32)
                    pieces = (lhsT_hi, lhsT_lo) if USE_LO else (lhsT_hi,)
                    for h in range(2):          # psum partition half
                        t = h * (G // 2) + s    # host vocab-tile index
                        i = 0
                        n_mm = len(pieces) * KB
                        for lhsT in pieces:
                            for kb in range(KB):
                                nc.tensor.matmul(
                                    ps[h * B:(h + 1) * B, :],
                                    lhsT[:, kb * B:(kb + 1) * B],
                                    ct_sb[:, (t * KB + kb) * VT:
                                          (t * KB + kb + 1) * VT],
                                    start=(i == 0), stop=(i == n_mm - 1))
                                i += 1
                    sl = slice(s * VT, (s + 1) * VT)
                    # penalties rows are identical across batch: keep only
                    # psum partitions {0, 64} (batch row 0 of each half)
                    nc.scalar.copy(pen_sb[0:1, sl], ps[0:1, :])
                    nc.scalar.copy(pen_sb[64:65, sl], ps[64:65, :])
                    nc.vector.tensor_sub(mod_sb[:, sl], lg_sb[:, sl], ps[:])
                    # store each finished half-column slab right away
                    nc.gpsimd.dma_start(mod_d[gi, :, sl], mod_sb[:, sl])
                nc.scalar.dma_start(pen_d[gi],
                                    pen_sb[0:128:64, :])

    nc.compile()
    return nc


_NC_CACHE = None


def _get_nc():
    global _NC_CACHE
    if _NC_CACHE is None:
        _NC_CACHE = _build_program()
    return _NC_CACHE


def _preprocess(gate_logits, rule_ids, token_ids):
    """Host-side integer index preprocessing (sharding + binning)."""
    rule_ids = np.asarray(rule_ids)
    token_ids = np.asarray(token_ids)

    firing = np.zeros(R, dtype=np.float32)
    firing[rule_ids] = 1.0
    n_firing = max(firing.sum(), 1.0)
    fs = (-firing / np.float32(n_firing)).astype(np.float32)

    # 16-entry LUT: exact fp8(e4m3) encodings of the small integer counts
    f8_lut = np.arange(16).astype(ml_dtypes.float8_e4m3).view(np.uint8)
    shard = token_ids // VSHARD
    cts = []
    for c in range(NCORES):
        m = shard == c
        keys = (token_ids[m] - c * VSHARD) * np.int32(R) + rule_ids[m]
        cnt = np.bincount(keys, minlength=VSHARD * R)
        assert cnt.max() <= 15, "count exceeds exact fp8 range"
        cnt = cnt.astype(np.uint8)
        # [v, r] -> [group][p][vt-in-group][kb][vv], rule r = p*KB + kb
        arr = f8_lut[cnt.reshape(NG, G, VT, 128, KB)]  # [g, t, vv, p, kb]
        arr = arr.transpose(0, 3, 1, 4, 2)             # [g, p, t, kb, vv]
        cts.append(np.ascontiguousarray(arr)
                   .reshape(NG, 128, G * KB * VT)
                   .view(ml_dtypes.float8_e4m3))
    return fs, cts


def _stack(shard):
    """[64, VSHARD] -> [NG, 128, G*VT//2] (partition p = h*64+b)."""
    hw = G * VT // 2
    return np.ascontiguousarray(
        shard.reshape(B, NG, 2, hw).transpose(1, 2, 0, 3).reshape(NG, 128, hw))


def _unstack(arr):
    """[NG, 128, G*VT//2] -> [64, VSHARD]."""
    hw = G * VT // 2
    return arr.reshape(NG, 2, B, hw).transpose(2, 0, 1, 3).reshape(B, VSHARD)


def build_in_maps(logits, gate_logits, rule_ids, token_ids):
    logits = np.asarray(logits, dtype=np.float32)
    gl = np.asarray(gate_logits, dtype=np.float32)
    fs, cts = _preprocess(gate_logits, rule_ids, token_ids)
    return [{
        "gl": gl,
        "fs": fs,
        "ct": cts[c],
        "logits": _stack(logits[:, c * VSHARD:(c + 1) * VSHARD]),
    } for c in range(NCORES)]


def kernel(logits, gate_logits, rule_ids, token_ids):
    in_maps = build_in_maps(logits, gate_logits, rule_ids, token_ids)
    nc = _get_nc()
    res = run_bass_kernel_spmd(nc, in_maps, list(range(NCORES)))

    modified = np.concatenate(
        [_unstack(r["mod"]) for r in res.results], axis=1)
    # device writes pen_1d once per core (rows are bit-identical across
    # batch, both here and in the reference); replicate on the host
    pen_1d = np.concatenate(
        [r["pen"].reshape(VSHARD) for r in res.results])
    penalties = np.ascontiguousarray(
        np.broadcast_to(pen_1d[None, :], (B, V)))
    coverage = np.float32(res.results[0]["cov"][0])
    return modified, coverage, penalties


# revision 35
# speedup vs baseline: 1.1061x; 1.1061x over previous
"""Trainium2 Bass kernel for DifferentiableLogicLayer (scatter_memory).

Math (see reference):
    gates    = sigmoid(gate_logits)                       [512]
    pen_1d   = scatter_add over E=1M pairs: 0.5*gates[rule] at token  [V]
    modified = logits - pen_1d[None, :]
    coverage = -(gates * firing).sum() / max(n_firing, 1)

Device strategy: the scatter-add is re-expressed as a dense matvec.
The (rule, token) pairs are binned on the host (pure integer index
preprocessing, per the vocab-shard sharding hint) into a count matrix
C[rule, v]; counts are tiny integers, exact in fp8.  Each of the 8
cores owns a 16000-wide vocab shard and computes

    pen[v] = sum_r C[r, v] * (0.5 * sigmoid(gate_logits[r]))

on the PE as 8 accumulating matmuls per 500-wide vocab tile (4 K-blocks
of 128 rules x {hi, lo} fp16 split of the gates for fp32-level
accuracy), with the 64 identical lhsT columns providing the batch
broadcast for free.  DVE then forms modified = logits - pen and the
penalties copy; outputs stream back over the ACT HWDGE queue.
"""
import numpy as np
import ml_dtypes

import concourse.bacc as bacc
import concourse.tile as tile
import concourse.mybir as mybir
from concourse.bass_utils import run_bass_kernel_spmd

F32 = mybir.dt.float32
F16 = mybir.dt.float16
F8 = mybir.dt.float8e4

B = 64             # batch
V = 128000         # vocab
R = 512            # rules
NCORES = 8
VSHARD = V // NCORES   # 16000 vocab per core
import os
VT = 500           # vocab tile (psum free dim)
G = int(os.environ.get("KERNEL_G", "4"))   # vocab tiles per DMA group
NG = VSHARD // (VT * G)   # groups per core
# 1: add the fp16 "lo" residual matmul stream (error ~3e-7 rel); 0: fp16
# gates only (error ~3e-5 rel, half the PE work).
USE_LO = os.environ.get("KERNEL_LO", "1") == "1"
KB = R // 128      # 4 rule blocks of 128

PENALTY_LAMBDA = 0.5


def _build_program():
    nc = bacc.Bacc("TRN2", target_bir_lowering=False, debug=False,
                   num_devices=NCORES)

    gl_d = nc.dram_tensor("gl", [R], F32, kind="ExternalInput").ap()
    fs_d = nc.dram_tensor("fs", [R], F32, kind="ExternalInput").ap()
    # counts, host-laid-out as [group][partition][vt-in-group][kb][v]
    ct_d = nc.dram_tensor("ct", [NG, 128, G * KB * VT], F8,
                          kind="ExternalInput").ap()
    # logits/outputs in host-stacked layout [group][128][half-width]:
    # partition p = h*64+b holds batch b, cols of half h of the group.
    logits_d = nc.dram_tensor("logits", [NG, 128, G * VT // 2], F32,
                              kind="ExternalInput").ap()
    mod_d = nc.dram_tensor("mod", [NG, 128, G * VT // 2], F32,
                           kind="ExternalOutput").ap()
    pen_d = nc.dram_tensor("pen", [NG, 2, G * VT // 2], F32,
                           kind="ExternalOutput").ap()
    cov_d = nc.dram_tensor("cov", [1], F32, kind="ExternalOutput").ap()

    with tile.TileContext(nc) as tc:
        with (
            tc.tile_pool(name="const", bufs=1) as const_pool,
            tc.tile_pool(name="cts", bufs=6) as ct_pool,
            tc.tile_pool(name="lgt", bufs=6) as lgt_pool,
            tc.tile_pool(name="outs", bufs=3) as out_pool,
            tc.tile_pool(name="psum", bufs=8, space="PSUM") as psum_pool,
        ):
            # ---- tiny parameter loads first (they gate the gates-prep chain
            # that every matmul depends on), then the big streaming prefetch.
            gl_sb = const_pool.tile([128, KB], F32)
            nc.sync.dma_start(gl_sb[:], gl_d.rearrange("(p k) -> p k", p=128))
            glr_sb = const_pool.tile([1, R], F32)
            nc.sync.dma_start(glr_sb[:], gl_d.rearrange("(o r) -> o r", o=1))
            fsr_sb = const_pool.tile([1, R], F32)
            nc.sync.dma_start(fsr_sb[:], fs_d.rearrange("(o r) -> o r", o=1))

            # Stacked layout: a group covers G*VT vocab cols; cols are split
            # into halves X|Y. SBUF partitions 0-63 hold (batch, X-cols),
            # 64-127 hold (batch, Y-cols) so DMA/DVE run all 128 partitions.
            GW = G * VT       # group width in vocab cols
            HW_ = GW // 2     # half width

            ct_tiles = {}
            lg_tiles = {}

            def issue_loads(gi):
                ct_sb = ct_pool.tile([128, G * KB * VT], F8, tag="ct")
                nc.sync.dma_start(ct_sb[:], ct_d[gi])
                lg_sb = lgt_pool.tile([128, HW_], F32, tag="lg")
                nc.scalar.dma_start(lg_sb[:], logits_d[gi])
                ct_tiles[gi] = ct_sb
                lg_tiles[gi] = lg_sb

            for gi in range(min(4, NG)):
                issue_loads(gi)

            # ---- gates prep: rule r lives at (partition p, col k), r = p*KB+k
            g_sb = const_pool.tile([128, KB], F32)
            nc.scalar.activation(g_sb[:], gl_sb[:],
                                 mybir.ActivationFunctionType.Sigmoid)
            g2_sb = const_pool.tile([128, KB], F32)
            nc.vector.tensor_scalar_mul(g2_sb[:], g_sb[:], PENALTY_LAMBDA)
            hi16 = const_pool.tile([128, KB], F16)
            nc.vector.tensor_copy(hi16[:], g2_sb[:])
            hi32 = const_pool.tile([128, KB], F32)
            nc.vector.tensor_copy(hi32[:], hi16[:])
            lo32 = const_pool.tile([128, KB], F32)
            nc.vector.tensor_sub(lo32[:], g2_sb[:], hi32[:])

            ones = const_pool.tile([128, B], F32)
            nc.vector.memset(ones[:], 1.0)
            lhsT_hi = const_pool.tile([128, KB * B], F16)
            lhsT_lo = const_pool.tile([128, KB * B], F16)
            for kb in range(KB):
                nc.vector.tensor_scalar_mul(
                    lhsT_hi[:, kb * B:(kb + 1) * B], ones[:],
                    hi32[:, kb:kb + 1])
                nc.vector.tensor_scalar_mul(
                    lhsT_lo[:, kb * B:(kb + 1) * B], ones[:],
                    lo32[:, kb:kb + 1])

            # ---- coverage loss (tiny; every core computes it)
            gr_sb = const_pool.tile([1, R], F32)
            nc.scalar.activation(gr_sb[:], glr_sb[:],
                                 mybir.ActivationFunctionType.Sigmoid)
            prod_sb = const_pool.tile([1, R], F32)
            nc.vector.tensor_mul(prod_sb[:], gr_sb[:], fsr_sb[:])
            cov_sb = const_pool.tile([1, 1], F32)
            nc.vector.tensor_reduce(cov_sb[:], prod_sb[:],
                                    mybir.AxisListType.X, mybir.AluOpType.add)
            nc.scalar.dma_start(cov_d, cov_sb[0, :])

            # ---- main loop over vocab groups
            for gi in range(NG):
                if gi + 4 < NG:
                    issue_loads(gi + 4)
                ct_sb = ct_tiles.pop(gi)
                lg_sb = lg_tiles.pop(gi)

                mod_sb = out_pool.tile([128, HW_], F32, tag="mod")
                pen_sb = out_pool.tile([128, HW_], F32, tag="pen")
                nhalf = G // 2
                for s in range(nhalf):
                    ps = psum_pool.tile([128, VT], F32)
                    pieces = (lhsT_hi, lhsT_lo) if USE_LO else (lhsT_hi,)
                    for h in range(2):          # psum partition half
                        t = h * (G // 2) + s    # host vocab-tile index
                        i = 0
                        n_mm = len(pieces) * KB
                        for lhsT in pieces:
                            for kb in range(KB):
                                nc.tensor.matmul(
                                    ps[h * B:(h + 1) * B, :],
                                    lhsT[:, kb * B:(kb + 1) * B],
                                    ct_sb[:, (t * KB + kb) * VT:
                                          (t * KB + kb + 1) * VT],
                                    start=(i == 0), stop=(i == n_mm - 1))
                                i += 1
                    sl = slice(s * VT, (s + 1) * VT)
                    # penalties rows are identical across batch: keep only
                    # psum partitions {0, 64} (batch row 0 of each half)
                    nc.scalar.copy(pen_sb[0:1, sl], ps[0:1, :])
                    nc.scalar.copy(pen_sb[64:65, sl], ps[64:65, :])
                    nc.vector.tensor_sub(mod_sb[:, sl], lg_sb[:, sl], ps[:])
                    # store each finished half-column slab right away
                    nc.gpsimd.dma_start(mod_d[gi, :, sl], mod_sb[:, sl])
                nc.scalar.dma_start(pen_d[gi],
                                    pen_sb[0:128:64, :])

    nc.compile()
    return nc


_NC_CACHE = None


def _get_nc():
    global _NC_CACHE
    if _NC_CACHE is None:
        _NC_CACHE = _build_program()
    return _NC_CACHE


def _preprocess(gate_logits, rule_ids, token_ids):
    """Host-side integer index preprocessing (sharding + binning)."""
    rule_ids = np.asarray(rule_ids)
    token_ids = np.asarray(token_ids)

    firing = np.zeros(R, dtype=np.float32)
    firing[rule_ids] = 1.0
    n_firing = max(firing.sum(), 1.0)
    fs = (-firing / np.float32(n_firing)).astype(np.float32)

    # 16-entry LUT: exact fp8(e4m3) encodings of the small integer counts
    f8_lut = np.arange(16).astype(ml_dtypes.float8_e4m3).view(np.uint8)
    shard = token_ids // VSHARD
    cts = []
    for c in range(NCORES):
        m = shard == c
        keys = (token_ids[m] - c * VSHARD) * np.int32(R) + rule_ids[m]
        cnt = np.bincount(keys, minlength=VSHARD * R)
        assert cnt.max() <= 15, "count exceeds exact fp8 range"
        cnt = cnt.astype(np.uint8)
        # [v, r] -> [group][p][vt-in-group][kb][vv], rule r = p*KB + kb
        arr = f8_lut[cnt.reshape(NG, G, VT, 128, KB)]  # [g, t, vv, p, kb]
        arr = arr.transpose(0, 3, 1, 4, 2)             # [g, p, t, kb, vv]
        cts.append(np.ascontiguousarray(arr)
                   .reshape(NG, 128, G * KB * VT)
                   .view(ml_dtypes.float8_e4m3))
    return fs, cts


def _stack(shard):
    """[64, VSHARD] -> [NG, 128, G*VT//2] (partition p = h*64+b)."""
    hw = G * VT // 2
    return np.ascontiguousarray(
        shard.reshape(B, NG, 2, hw).transpose(1, 2, 0, 3).reshape(NG, 128, hw))


def _unstack(arr):
    """[NG, 128, G*VT//2] -> [64, VSHARD]."""
    hw = G * VT // 2
    return arr.reshape(NG, 2, B, hw).transpose(2, 0, 1, 3).reshape(B, VSHARD)


def build_in_maps(logits, gate_logits, rule_ids, token_ids):
    logits = np.asarray(logits, dtype=np.float32)
    gl = np.asarray(gate_logits, dtype=np.float32)
    fs, cts = _preprocess(gate_logits, rule_ids, token_ids)
    return [{
        "gl": gl,
        "fs": fs,
        "ct": cts[c],
        "logits": _stack(logits[:, c * VSHARD:(c + 1) * VSHARD]),
    } for c in range(NCORES)]


def kernel(logits, gate_logits, rule_ids, token_ids):
    in_maps = build_in_maps(logits, gate_logits, rule_ids, token_ids)
    nc = _get_nc()
    res = run_bass_kernel_spmd(nc, in_maps, list(range(NCORES)))

    modified = np.concatenate(
        [_unstack(r["mod"]) for r in res.results], axis=1)
    # device writes pen_1d once per core (rows are bit-identical across
    # batch, both here and in the reference); replicate on the host
    pen_1d = np.concatenate(
        [r["pen"].reshape(VSHARD) for r in res.results])
    penalties = np.ascontiguousarray(
        np.broadcast_to(pen_1d[None, :], (B, V)))
    coverage = np.float32(res.results[0]["cov"][0])
    return modified, coverage, penalties
